# revision 1
# baseline (speedup 1.0000x reference)
"""Soft-DTW layer (band-limited, gamma=1) as a Bass/Tile kernel on 8 TRN2 cores.

Problem: x [64, 512] f32, protos [32, 64] f32 -> out [64, 32, 1] f32
  out[b, f, 0] = softDTW(C[b,f]) / T, C[b,f][i,j] = (x[b,i]-protos[f,j])^2,
  Sakoe-Chiba band |i/511 - j/63| <= 0.2, out-of-band = BIG.

Algorithm (per (b,f) problem, exp-space):
  E'(i,j) = e^{a*i - D(i,j)} satisfies, with G = e^{a-C} (0 outside band),
    E'(i,j) = G(i,j) * (E'(i-1,j) + E'(i-1,j-1) + e^{-a} * E'(i,j-1))
  Column sweep j=0..63; column j's in-band rows live in the window
  [8j-104, 8j+112) (216 rows). Per column, on the Vector engine:
    w[v]  = ECA*cprev[8+v] + cprev[7+v]          (scalar_tensor_tensor)
    E[v]  = (w[v] + E[v-1]) * G[v]               (tensor_tensor_scan add,mult)
  Every FB columns a per-problem rescale s=1/max keeps f32 range; log(max)
  is accumulated and added back at the end.

  G production is pipelined ahead of the DP in column chunks, entirely off
  the Vector engine: ACT computes (x - p_j)^2 via Square with per-partition
  bias -p_j (no subtract op), then exp(A - C) per chunk; GpSimd zeroes the
  few out-of-band edge cells per column (exact band enforcement). Padded x
  (XBIG) makes out-of-range rows underflow to G=0 in the exp.

Sharding: data-parallel over batch. Core c handles b in [8c, 8c+8); its 256
(b,f) problems sit as 2 groups of 128 partitions:
  partition p, group g -> b = 8c + 4g + p//32, f = p%32.
"""

import numpy as np

import concourse.bass as bass
import concourse.bacc as bacc
import concourse.mybir as mybir
import concourse.tile as tile
from concourse.bass_utils import run_bass_kernel_spmd

T, K = 512, 64
NCORES = 8
L = 216           # column window length
GC = 2 * L        # per-column G payload (both groups)
GS = 448          # per-column G stride (448 f32 = 1792 B, 128B-aligned)
CS = GC + 16      # column buffer: [g0 216 | g1 216 | 16 zero tail]
BOFF = 104        # column j covers rows [8j-104, 8j+112)
XPAD = BOFF + T + 112   # padded x row length (728)
XBIG = 1.0e4      # pad value; (XBIG-p)^2 ~ 1e8 -> exp -> 0
A = 0.75          # rescale slope per row
FB = 8            # renorm every FB columns
ECA = float(np.exp(-A))
F32 = mybir.dt.float32
AL = mybir.AluOpType

# chunk sizes for the G pipeline (sum = K); gradual ramp so ACT's G
# production stays ahead of the DVE consumer through the pipeline fill
CHUNKS = [1, 2, 4, 6, 8, 8, 8, 8, 8, 11]


def _band_runs():
    """Per column: (a, b) with in-band window cells exactly [a, b); cells
    [0,a) and [b,216) must have G=0.  Band mask computed bit-identically to
    the reference (float32 arithmetic)."""
    ii = np.arange(T, dtype=np.float32) / np.float32(T - 1)
    jj = np.arange(K, dtype=np.float32) / np.float32(K - 1)
    mask = np.abs(ii[:, None] - jj[None, :]) <= np.float32(0.2)
    runs = []
    for j in range(K):
        win0 = 8 * j - BOFF
        rows = win0 + np.arange(L)
        inb = np.zeros(L, bool)
        valid = (rows >= 0) & (rows < T)
        inb[valid] = mask[rows[valid], j]
        idx = np.where(inb)[0]
        runs.append((int(idx[0]), int(idx[-1]) + 1))
    return runs


BAND = _band_runs()


def _ap(t, offset, dims):
    """Custom free-dim access pattern on tile t: dims = [[step, count], ...]
    (element units), keeping the partition dim."""
    ap = t[:, 0:1].copy()
    ap.ap = ap.ap[:1] + [[int(s), int(n)] for s, n in dims]
    ap.offset = int(offset)
    return ap


def build_nc():
    nc = bacc.Bacc("TRN2")
    xs = nc.dram_tensor("xs", [8, T], F32, kind="ExternalInput")
    pr = nc.dram_tensor("protos", [32, K], F32, kind="ExternalInput")
    out = nc.dram_tensor("out", [128, 2], F32, kind="ExternalOutput")

    nfb = sum(1 for j in range(K) if j % FB == 0 and j > 0)  # renorm events

    with tile.TileContext(nc) as tc:
        with tc.tile_pool(name="main", bufs=1) as pool, \
                tc.psum_pool(name="xps", bufs=1) as ppool:
            # x lives in PSUM: xs is DMA'd once (16 KB) into 8 partitions,
            # then a selection-matrix matmul replicates it to the
            # (partition p, group g) -> row 4g + p//32 layout.  ACT Square
            # reads PSUM directly; no padded SBUF copy of x exists.
            xs8 = pool.tile([8, T], F32)             # raw x rows
            Wt = pool.tile([8, 256], F32)            # selection weights (2 groups)
            P = ppool.tile([128, 2 * T], F32)        # replicated x (2 banks)
            prt = pool.tile([128, K], F32)           # protos row per problem
            negp = pool.tile([128, K], F32)          # -protos (Square bias)
            # banded G, one tile per chunk (exact RAW/WAR tracking), layout
            # (j, g, v) with a 16-elem alignment pad per column
            Gt = [pool.tile([128, W * GS], F32, name=f"G{k}")
                  for k, W in enumerate(CHUNKS)]
            colA = pool.tile([128, CS], F32)
            colB = pool.tile([128, CS], F32)
            w = pool.tile([128, GC], F32)
            mxb = pool.tile([128, 2 * nfb], F32)     # renorm scales (k, g)
            s2 = pool.tile([128, 2], F32)
            lnmx = pool.tile([128, 2 * nfb], F32)    # ln of scales
            ef = pool.tile([128, 2], F32)
            efe = pool.tile([128, 2], mybir.dt.int32)
            eff = pool.tile([128, 2], F32)
            efm = pool.tile([128, 2], mybir.dt.int32)
            lnmant = pool.tile([128, 2], F32)
            lnef = pool.tile([128, 2], F32)
            lnS = pool.tile([128, 2], F32)
            tt = pool.tile([128, 2], F32)
            osb = pool.tile([128, 2], F32)
            acon = pool.tile([128, 1], F32)          # bias const A for Exp
            scr = pool.tile([128, 24], F32)          # DVE pre-touch scratch

            # ---- init: DMAs first so transfer overlaps the DVE memsets ----
            nc.sync.dma_start(xs8[:, :], xs[:, :])
            # protos: DRAM [32, 64] -> partition p reads row p%32
            psrc = pr[:, :].unsqueeze(0).broadcast_to([4, 32, K])
            nc.sync.dma_start(prt[:, :], psrc)
            # selection weights: W[i, 128g + m] = 1 iff i == 4g + m//32
            # W[p, c] = 1 iff c in [32p, 32p+32)  (equals the (g, m) selection
            # W[i, 128g + m] = 1 iff i == 4g + m//32 for the 8 used rows)
            nc.vector.memset(Wt[:, :], 1.0)
            nc.gpsimd.affine_select(
                Wt[:, :], Wt[:, :], pattern=[[1, 256]], base=0,
                compare_op=AL.is_ge, fill=0.0, channel_multiplier=-32)
            nc.gpsimd.affine_select(
                Wt[:, :], Wt[:, :], pattern=[[-1, 256]], base=31,
                compare_op=AL.is_ge, fill=0.0, channel_multiplier=32)
            nc.vector.memset(colA[:, :], 0.0)
            nc.vector.memset(colB[:, :], 0.0)
            nc.vector.memset(w[:, :], 0.0)
            # virtual-corner seed E'(-1,-1)=e^{-a} at row -1 of column -1
            # (column -1 window starts at row -112; row -1 -> pos 111)
            nc.vector.memset(colA[:, 111:112], ECA)
            nc.vector.memset(colA[:, L + 111:L + 112], ECA)
            nc.vector.memset(acon[:, :], A)
            nc.vector.tensor_copy(scr[:, 2:3], prt[:, 0:1])
            nc.vector.tensor_scalar(negp[:, :], prt[:, :], -1.0, None, op0=AL.mult)
            nc.vector.tensor_copy(scr[0:8, 0:1], xs8[:, 0:1])  # touch x DMA
            tc.no_sync_barrier()
            # replicate x into PSUM: P[p, g*T + t] = xs[4g + p//32, t]
            nc.tensor.matmul(P[:, 0:T], Wt[:, 0:128], xs8[:, :])
            nc.tensor.matmul(P[:, T:2 * T], Wt[:, 128:256], xs8[:, :])
            # ACT relay covers negp for all squares; the squares' PSUM reads
            # wait on TensorE directly.
            nc.vector.tensor_copy(scr[:, 4:5], negp[:, 0:1])     # DVE stamp
            nc.scalar.copy(scr[:, 5:6], scr[:, 4:5])             # ACT sees DVE
            tc.no_sync_barrier()

            # ---- pipelined G production + column DP ----
            fb_k = 0
            cprev, ccur = colA, colB
            j0 = 0
            for ck, W in enumerate(CHUNKS):
                cols = range(j0, j0 + W)
                G = Gt[ck]
                # ACT: C = (x - p_j)^2 per column (Square with bias -p_j),
                # reading x straight from PSUM, in-band cells only (the PSUM
                # rows [0,T) exist; out-of-band/out-of-range G cells are
                # zeroed by the pool memsets below)
                for j in cols:
                    a, b = BAND[j]
                    gout = _ap(G, (j - j0) * GS + a, [[L, 2], [1, b - a]])
                    xin = _ap(P, 8 * j - BOFF + a, [[T, 2], [1, b - a]])
                    nc.scalar.activation(gout, xin,
                                         mybir.ActivationFunctionType.Square,
                                         bias=negp[:, j:j + 1], scale=1.0)
                # ACT: G = exp(A - C) for the whole chunk, in place
                # (strided AP skips the 16-elem alignment pad per column)
                gch = _ap(G, 0, [[GS, W], [1, GC]])
                nc.scalar.activation(gch, gch,
                                     mybir.ActivationFunctionType.Exp,
                                     bias=acon[:, :], scale=-1.0)
                # Pool: zero out-of-band edge cells (exact band enforcement).
                # Per column (a,b): zero [b, 216+a) (g0 tail + g1 lead merged)
                # and across columns [216+b_j, 448) + next col's [0, a_{j+1}).
                for j in cols:
                    a, b = BAND[j]
                    base = (j - j0) * GS
                    if j == j0 and a > 0:
                        nc.gpsimd.memset(G[:, base:base + a], 0.0)
                    nc.gpsimd.memset(G[:, base + b:base + L + a], 0.0)
                    if j < j0 + W - 1:
                        a2 = BAND[j + 1][0]
                        nc.gpsimd.memset(G[:, base + L + b:base + GS + a2], 0.0)
                    else:
                        nc.gpsimd.memset(G[:, base + L + b:base + GC], 0.0)
                # (No flag relay: the tile scheduler reorders by dependency,
                # so each scan carries one direct pool-counter wait, which
                # transitively covers the ACT exp via the memsets' own wait.)

                # DVE: column DP for this chunk.  The STT only computes w on
                # the in-band cells [a, be) of each group (2-free-dim AP);
                # cells outside keep stale values, which the scan kills via
                # G=0 (x*0=0, and w was zero-initialized so never NaN).
                # be additionally clips to the dependency cone of the output
                # cell (row 511 of column 63): e = 112 + 8*(63-j).  The scan
                # runs [a, L+be): it must cover all of g0 (the G=0 cells at
                # [b, L) reset the carry before g1) but can skip g0's head
                # and g1's tail; the next column's shifted reads stay inside
                # the freshly written range by construction.
                for j in cols:
                    a, b = BAND[j]
                    e = min(L, 112 + 8 * (63 - j))   # cone end, exclusive
                    be = min(b, e)
                    bw = be - a
                    gcol = G[:, (j - j0) * GS + a:(j - j0) * GS + L + e]
                    w_ap = _ap(w, a, [[L, 2], [1, bw]])
                    d_ap = _ap(cprev, 8 + a, [[L, 2], [1, bw]])
                    l_ap = _ap(cprev, 7 + a, [[L, 2], [1, bw]])
                    if j % FB == 0 and j > 0:
                        # renorm: per-group scale 1/sum(w) via accum_out
                        sl = mxb[:, 2 * fb_k:2 * fb_k + 2]
                        nc.vector.scalar_tensor_tensor(
                            w[:, a:be], cprev[:, 8 + a:8 + be], ECA,
                            cprev[:, 7 + a:7 + be],
                            op0=AL.mult, op1=AL.add,
                            accum_out=sl[:, 0:1])
                        nc.vector.scalar_tensor_tensor(
                            w[:, L + a:L + be], cprev[:, 8 + L + a:8 + L + be],
                            ECA, cprev[:, 7 + L + a:7 + L + be],
                            op0=AL.mult, op1=AL.add,
                            accum_out=sl[:, 1:2])
                        nc.vector.reciprocal(s2[:, :], sl)
                        nc.vector.tensor_scalar(w[:, a:be], w[:, a:be],
                                                s2[:, 0:1], None, op0=AL.mult)
                        nc.vector.tensor_scalar(w[:, L + a:L + be],
                                                w[:, L + a:L + be],
                                                s2[:, 1:2], None, op0=AL.mult)
                        fb_k += 1
                    else:
                        nc.vector.scalar_tensor_tensor(
                            w_ap, d_ap, ECA, l_ap,
                            op0=AL.mult, op1=AL.add)
                    nc.vector.tensor_tensor_scan(
                        ccur[:, a:L + e], w[:, a:L + e], gcol, 0.0,
                        op0=AL.add, op1=AL.mult)
                    cprev, ccur = ccur, cprev
                j0 += W
            # deferred renorm logs: one Ln over all stored scales (keeps the
            # ACT Exp/Ln tables from thrashing inside the loop).  The dummy
            # Ln is anchored on the last G tile so the scheduler keeps it
            # AFTER the exps; ACT then loads the Ln table in its idle window
            # instead of on the critical tail.
            nc.scalar.activation(scr[:, 7:8], Gt[-1][:, 0:1],
                                 mybir.ActivationFunctionType.Ln)
            nc.scalar.activation(lnmx[:, :], mxb[:, :],
                                 mybir.ActivationFunctionType.Ln)

            last = cprev  # column 63 buffer
            # ---- extraction: D = a*511 - sum(ln mx) - ln(E'fin); out = D/512 ----
            nc.vector.tensor_reduce(
                lnS[:, :], lnmx[:, :].rearrange("p (k g) -> p g k", g=2),
                axis=mybir.AxisListType.X, op=AL.add)
            nc.vector.tensor_copy(ef[:, 0:1], last[:, 111:112])
            nc.vector.tensor_copy(ef[:, 1:2], last[:, L + 111:L + 112])
            # ACT's Ln mishandles tiny args, so frexp-style log:
            # ln(ef) = Ln(mantissa) + (exp - 127)*ln2 (the -127*ln2 is folded
            # into the final affine)
            eiv = ef[:, :].bitcast(mybir.dt.int32)
            nc.vector.tensor_scalar(efe[:, :], eiv, 23, None,
                                    op0=AL.arith_shift_right)
            nc.vector.tensor_copy(eff[:, :], efe[:, :])   # int -> float value
            nc.vector.tensor_scalar(efm[:, :], eiv, 0x007FFFFF, 0x3F800000,
                                    op0=AL.bitwise_and,
                                    op1=AL.bitwise_or)
            nc.scalar.activation(lnmant[:, :], efm[:, :].bitcast(F32),
                                 mybir.ActivationFunctionType.Ln)
            nc.vector.scalar_tensor_tensor(
                lnef[:, :], eff[:, :], float(np.log(2.0)), lnmant[:, :],
                op0=AL.mult, op1=AL.add)
            nc.vector.tensor_tensor(tt[:, :], lnS[:, :], lnef[:, :],
                                    op=AL.add)
            nc.vector.tensor_scalar(
                osb[:, :], tt[:, :], float(-1.0 / T),
                float((A * (T - 1) + 127.0 * np.log(2.0)) / T),
                op0=AL.mult, op1=AL.add)
            nc.sync.dma_start(out[:, :], osb[:, :])

    nc.compile()
    return nc


_NC = None


def _get_nc():
    global _NC
    if _NC is None:
        _NC = build_nc()
    return _NC


def kernel(x: np.ndarray, protos: np.ndarray) -> np.ndarray:
    x = np.ascontiguousarray(x, dtype=np.float32)
    protos = np.ascontiguousarray(protos, dtype=np.float32)
    nc = _get_nc()
    in_maps = [
        {"xs": x[8 * c: 8 * c + 8], "protos": protos} for c in range(NCORES)
    ]
    res = run_bass_kernel_spmd(nc, in_maps, core_ids=list(range(NCORES)))
    out = np.empty((64, 32, 1), dtype=np.float32)
    for c in range(NCORES):
        r = res.results[c]["out"]                 # [128, 2]
        blk = r.reshape(4, 32, 2).transpose(2, 0, 1)  # [g, bb, f]
        out[8 * c: 8 * c + 8, :, 0] = blk.reshape(8, 32)
    return out



# revision 2
# speedup vs baseline: 1.2037x; 1.2037x over previous
"""Soft-DTW layer (band-limited, gamma=1) as a Bass/Tile kernel on 8 TRN2 cores.

Problem: x [64, 512] f32, protos [32, 64] f32 -> out [64, 32, 1] f32
  out[b, f, 0] = softDTW(C[b,f]) / T, C[b,f][i,j] = (x[b,i]-protos[f,j])^2,
  Sakoe-Chiba band |i/511 - j/63| <= 0.2, out-of-band = BIG.

Exp-space DP (per (b,f) problem): E'(i,j) = e^{a*i - D(i,j)} satisfies, with
G = e^{a-C},
    E'(i,j) = G(i,j) * (E'(i-1,j) + E'(i-1,j-1) + e^{-a} * E'(i,j-1))
Column sweep j=0..63 over the in-band run [a_j, b_j) of the 216-row window
[8j-104, 8j+112).

v3 structure (three-engine pipeline, exact band with NO zeroing passes):
  PE:  w_j = ECA*E_{j-1}[8+v] + E_{j-1}[7+v] via two accumulating
       diagonal-matmuls (bf16 identity weights; the shift lives in the rhs
       access pattern) into PSUM.  Per-problem renorm scales ride the
       diagonals (scaled weight copies, rebuilt per renorm event).
  ACT: G production: per-column Square (bias -p_j) from PSUM-replicated x,
       per-chunk Exp; also the renorm magnitude sums (Copy+accum) and the
       scaled-weight rebuilds.
  DVE: per-column per-group scans E_j = scan(w_j(PSUM), G) -> bf16, over
       exactly the in-band run.  The two groups' chains interleave, so PE's
       w-matmuls hide behind the sibling group's scan.
Band edges come for free: scan initial=0 resets the left border, and the
monotone drift of (a_j, b_j) means all out-of-run taps land on never-written
zero-initialized cells (verified structurally).

Sharding: data-parallel over batch. Core c handles b in [8c, 8c+8); its 256
(b,f) problems sit as 2 groups of 128 partitions:
  partition p, group g -> b = 8c + 4g + p//32, f = p%32.
"""

import numpy as np

import concourse.bass as bass
import concourse.bacc as bacc
import concourse.mybir as mybir
import concourse.tile as tile
from concourse.bass_utils import run_bass_kernel_spmd

T, K, L = 512, 64, 216
NCORES = 8
BOFF = 104        # column j covers rows [8j-104, 8j+112)
GS = 448          # per-column G stride: [g0 216 | g1 216 | pad 16]
EW = 224          # per-group E-buffer width (216 + 8 tap slack)
A = 0.75          # rescale slope per row
FB = 8            # renorm every FB columns
ECA = float(np.exp(-A))
F32 = mybir.dt.float32
BF16 = mybir.dt.bfloat16
AL = mybir.AluOpType
AF = mybir.ActivationFunctionType

# chunk sizes for the G pipeline (sum = K); ramp keeps ACT ahead of DVE
CHUNKS = [1, 2, 4, 6, 8, 8, 8, 8, 8, 11]


def _band_runs():
    """Per column: (a, b) with in-band window cells exactly [a, b);
    bit-identical to the reference band mask (float32 arithmetic)."""
    ii = np.arange(T, dtype=np.float32) / np.float32(T - 1)
    jj = np.arange(K, dtype=np.float32) / np.float32(K - 1)
    mask = np.abs(ii[:, None] - jj[None, :]) <= np.float32(0.2)
    runs = []
    for j in range(K):
        rows = 8 * j - BOFF + np.arange(L)
        inb = np.zeros(L, bool)
        valid = (rows >= 0) & (rows < T)
        inb[valid] = mask[rows[valid], j]
        idx = np.where(inb)[0]
        runs.append((int(idx[0]), int(idx[-1]) + 1))
    return runs


BAND = _band_runs()
NFB = sum(1 for j in range(K) if j % FB == 0 and j > 0)


def _ap(t, offset, dims):
    """Custom free-dim access pattern on tile t: dims = [[step, count], ...]
    (element units), keeping the partition dim."""
    ap = t[:, 0:1].copy()
    ap.ap = ap.ap[:1] + [[int(s), int(n)] for s, n in dims]
    ap.offset = int(offset)
    return ap


def build_nc():
    nc = bacc.Bacc("TRN2")
    xs = nc.dram_tensor("xs", [8, T], F32, kind="ExternalInput")
    pr = nc.dram_tensor("protos", [32, K], F32, kind="ExternalInput")
    out = nc.dram_tensor("out", [128, 2], F32, kind="ExternalOutput")

    with tile.TileContext(nc) as tc:
        with tc.tile_pool(name="main", bufs=1) as pool, \
                tc.psum_pool(name="xps", bufs=1) as ppool:
            xs8 = pool.tile([8, T], F32)             # raw x rows
            Wt = pool.tile([8, 256], F32)            # x-replication weights
            P = ppool.tile([128, 2 * T], F32)        # replicated x (2 banks)
            prt = pool.tile([128, K], F32)           # protos row per problem
            negp = pool.tile([128, K], F32)          # -protos (Square bias)
            # banded G, one tile per chunk (exact RAW/WAR tracking)
            Gt = [pool.tile([128, W * GS], F32, name=f"G{k}")
                  for k, W in enumerate(CHUNKS)]
            # E buffers: both groups in one tile (g at offset 224*g), bf16
            colA = pool.tile([128, 2 * EW], BF16)
            colB = pool.tile([128, 2 * EW], BF16)
            # identity weights: base and per-group renorm-scaled copies
            Ib = pool.tile([128, 128], BF16)
            Ie = pool.tile([128, 128], BF16)
            Ibs = [pool.tile([128, 128], BF16, name=f"Ibs{g}") for g in (0, 1)]
            Ies = [pool.tile([128, 128], BF16, name=f"Ies{g}") for g in (0, 1)]
            # w in PSUM: (group, ping) -> tile
            wps = [ppool.tile([128, 208], F32, name=f"w{i}") for i in range(4)]
            sl = pool.tile([128, 2], F32)            # renorm sums (g)
            mxb = pool.tile([128, 2 * NFB], F32)     # applied scales s (k, g)
            lnmx = pool.tile([128, 2 * NFB], F32)    # ln of scales
            rsc = pool.tile([128, L], F32)           # renorm-sum copy target
            ef = pool.tile([128, 2], F32)
            efe = pool.tile([128, 2], mybir.dt.int32)
            eff = pool.tile([128, 2], F32)
            efm = pool.tile([128, 2], mybir.dt.int32)
            lnmant = pool.tile([128, 2], F32)
            lnef = pool.tile([128, 2], F32)
            lnS = pool.tile([128, 2], F32)
            tt = pool.tile([128, 2], F32)
            osb = pool.tile([128, 2], F32)
            acon = pool.tile([128, 1], F32)          # bias const A for Exp
            scr = pool.tile([128, 8], F32)           # scratch

            # ---- init ----
            nc.sync.dma_start(xs8[:, :], xs[:, :])
            psrc = pr[:, :].unsqueeze(0).broadcast_to([4, 32, K])
            nc.sync.dma_start(prt[:, :], psrc)
            # x-replication weights: W[p, 128g + m] = 1 iff p == 4g + m//32
            nc.vector.memset(Wt[:, :], 1.0)
            nc.gpsimd.affine_select(
                Wt[:, :], Wt[:, :], pattern=[[1, 256]], base=0,
                compare_op=AL.is_ge, fill=0.0, channel_multiplier=-32)
            nc.gpsimd.affine_select(
                Wt[:, :], Wt[:, :], pattern=[[-1, 256]], base=31,
                compare_op=AL.is_ge, fill=0.0, channel_multiplier=32)
            # identity (diagonal) weights
            nc.vector.memset(Ib[:, :], 1.0)
            nc.gpsimd.affine_select(
                Ib[:, :], Ib[:, :], pattern=[[1, 128]], base=0,
                compare_op=AL.is_ge, fill=0.0, channel_multiplier=-1)
            nc.gpsimd.affine_select(
                Ib[:, :], Ib[:, :], pattern=[[-1, 128]], base=0,
                compare_op=AL.is_ge, fill=0.0, channel_multiplier=1)
            nc.vector.tensor_scalar(Ie[:, :], Ib[:, :], ECA, None, op0=AL.mult)
            nc.vector.memset(colA[:, :], 0.0)
            nc.vector.memset(colB[:, :], 0.0)
            # virtual-corner seed E'(-1,-1) = e^{-a} at window position 111
            nc.vector.memset(colA[:, 111:112], ECA)
            nc.vector.memset(colA[:, EW + 111:EW + 112], ECA)
            nc.vector.memset(acon[:, :], A)
            nc.vector.tensor_scalar(negp[:, :], prt[:, :], -1.0, None,
                                    op0=AL.mult)
            nc.vector.tensor_copy(scr[0:8, 0:1], xs8[:, 0:1])  # touch x DMA
            tc.no_sync_barrier()
            # replicate x into PSUM: P[p, g*T + t] = xs[4g + p//32, t]
            nc.tensor.matmul(P[:, 0:T], Wt[:, 0:128], xs8[:, :])
            nc.tensor.matmul(P[:, T:2 * T], Wt[:, 128:256], xs8[:, :])
            tc.no_sync_barrier()

            # ---- pipelined G production + column DP ----
            fb_k = 0
            cprev, ccur = colA, colB
            j0 = 0
            for ck, W in enumerate(CHUNKS):
                cols = range(j0, j0 + W)
                G = Gt[ck]
                # ACT: C = (x - p_j)^2, in-band cells of both groups
                for j in cols:
                    a, b = BAND[j]
                    gout = _ap(G, (j - j0) * GS + a, [[L, 2], [1, b - a]])
                    xin = _ap(P, 8 * j - BOFF + a, [[T, 2], [1, b - a]])
                    nc.scalar.activation(gout, xin, AF.Square,
                                         bias=negp[:, j:j + 1], scale=1.0)
                # ACT: G = exp(A - C) for the whole chunk (out-of-run cells
                # hold garbage-exp values; nothing ever reads them)
                gch = _ap(G, 0, [[GS, W], [1, 2 * L]])
                nc.scalar.activation(gch, gch, AF.Exp,
                                     bias=acon[:, :], scale=-1.0)

                for j in cols:
                    a, b = BAND[j]
                    n = b - a
                    renorm = (j % FB == 0 and j > 0)
                    if renorm:
                        # scale from column j-2's magnitude (off critical
                        # path): ccur still holds E_{j-2}
                        ap2, bp2 = BAND[j - 2]
                        for g in (0, 1):
                            nc.scalar.activation(
                                rsc[:, 0:bp2 - ap2],
                                ccur[:, EW * g + ap2:EW * g + bp2],
                                AF.Copy, accum_out=sl[:, g:g + 1])
                        nc.vector.reciprocal(
                            mxb[:, 2 * fb_k:2 * fb_k + 2], sl[:, :])
                        for g in (0, 1):
                            sap = mxb[:, 2 * fb_k + g:2 * fb_k + g + 1]
                            nc.scalar.mul(Ibs[g][:, :], Ib[:, :], sap)
                            nc.scalar.mul(Ies[g][:, :], Ie[:, :], sap)
                        fb_k += 1
                    for g in (0, 1):
                        wp = wps[2 * g + (j % 2)]
                        iu, eu = (Ibs[g], Ies[g]) if renorm else (Ib, Ie)
                        o = EW * g
                        nc.tensor.matmul(wp[:, 0:n], eu[:, :],
                                         cprev[:, o + 8 + a:o + 8 + b],
                                         start=True, stop=False)
                        nc.tensor.matmul(wp[:, 0:n], iu[:, :],
                                         cprev[:, o + 7 + a:o + 7 + b],
                                         start=False, stop=True)
                        nc.vector.tensor_tensor_scan(
                            ccur[:, o + a:o + b], wp[:, 0:n],
                            G[:, (j - j0) * GS + L * g + a:
                               (j - j0) * GS + L * g + b],
                            0.0, op0=AL.add, op1=AL.mult)
                    cprev, ccur = ccur, cprev
                j0 += W

            # deferred renorm logs; dummy Ln anchored on the last G tile so
            # ACT loads the Ln table after the exps
            nc.scalar.activation(scr[:, 2:3], Gt[-1][:, 0:1], AF.Ln)
            nc.scalar.activation(lnmx[:, :], mxb[:, :], AF.Ln)

            last = cprev  # column 63 buffer
            # ---- extraction: D = a*511 + sum(ln s) - ln(E'fin); out = D/T --
            nc.vector.tensor_reduce(
                lnS[:, :], lnmx[:, :].rearrange("p (k g) -> p g k", g=2),
                axis=mybir.AxisListType.X, op=AL.add)
            nc.vector.tensor_copy(ef[:, 0:1], last[:, 111:112])
            nc.vector.tensor_copy(ef[:, 1:2], last[:, EW + 111:EW + 112])
            # frexp-style log: ln(ef) = Ln(mantissa) + (exp - 127)*ln2
            eiv = ef[:, :].bitcast(mybir.dt.int32)
            nc.vector.tensor_scalar(efe[:, :], eiv, 23, None,
                                    op0=AL.arith_shift_right)
            nc.vector.tensor_copy(eff[:, :], efe[:, :])   # int -> float value
            nc.vector.tensor_scalar(efm[:, :], eiv, 0x007FFFFF, 0x3F800000,
                                    op0=AL.bitwise_and,
                                    op1=AL.bitwise_or)
            nc.scalar.activation(lnmant[:, :], efm[:, :].bitcast(F32),
                                 AF.Ln)
            nc.vector.scalar_tensor_tensor(
                lnef[:, :], eff[:, :], float(np.log(2.0)), lnmant[:, :],
                op0=AL.mult, op1=AL.add)
            # tt = ln(E'fin) - lnS ; out = -tt/T + (A(T-1) + 127 ln2)/T
            nc.vector.tensor_tensor(tt[:, :], lnef[:, :], lnS[:, :],
                                    op=AL.subtract)
            nc.vector.tensor_scalar(
                osb[:, :], tt[:, :], float(-1.0 / T),
                float((A * (T - 1) + 127.0 * np.log(2.0)) / T),
                op0=AL.mult, op1=AL.add)
            nc.sync.dma_start(out[:, :], osb[:, :])

    nc.compile()
    return nc


_NC = None


def _get_nc():
    global _NC
    if _NC is None:
        _NC = build_nc()
    return _NC


def kernel(x: np.ndarray, protos: np.ndarray) -> np.ndarray:
    x = np.ascontiguousarray(x, dtype=np.float32)
    protos = np.ascontiguousarray(protos, dtype=np.float32)
    nc = _get_nc()
    in_maps = [
        {"xs": x[8 * c: 8 * c + 8], "protos": protos} for c in range(NCORES)
    ]
    res = run_bass_kernel_spmd(nc, in_maps, core_ids=list(range(NCORES)))
    out = np.empty((64, 32, 1), dtype=np.float32)
    for c in range(NCORES):
        r = res.results[c]["out"]                 # [128, 2]
        blk = r.reshape(4, 32, 2).transpose(2, 0, 1)  # [g, bb, f]
        out[8 * c: 8 * c + 8, :, 0] = blk.reshape(8, 32)
    return out
